# revision 18
# baseline (speedup 1.0000x reference)
"""Multi-head attention (AttnHeads) Trainium2 Bass kernel, 8-core SPMD.

Problem: x [4, 2048, 1024] fp32, qkv [1024, 3072] fp32, out_weight [1024, 1024] fp32.
  qkv_out = x @ qkv; q,k,v = split(qkv_out); heads n=16, d=64
  S_n = Q_n K_n^T (scores [t, s]); P = softmax_s(S); C_n = P_n @ V_n
  out = (sum_n C_n @ OW_n) / 8

Sharding: core c -> batch c//2, head-group c%2 (8 heads each). Each core
computes a partial [2048, 1024] output; host sums the two partials per batch.

Per-core pipeline (single NeuronCore, Tile-scheduled):
  A) load x^T / weight slices
  B) Q^T, K^T = W^T x^T   [dq=512, t=2048] fp32r (full fp32 precision at bf16 rate)
  C) V = x W_v            [s=2048, dv=512] bf16
  D) per (head, t-block 128): S=[128,2048] psum (fp32r matmul);
     DVE reduce_max(negate); ACT exp(bias=-max, accum_out=rowsum) -> E bf16;
     DMA-xbar transpose E -> E^T; PE: C^T += V^T E^T accumulated over s.
  E) normalize C by 1/rowsum (DMA transpose round-trip to get per-t axis onto
     partitions), then out-proj O = C @ OW accumulated over heads in PSUM.
"""

import numpy as np
import ml_dtypes

import concourse.bass as bass
import concourse.mybir as mybir
import concourse.tile as tile
from concourse.bass_utils import run_bass_kernel_spmd

NUM_HEADS = 16
HIDDEN = 1024
HEAD = 64
BATCH = 4
SEQ = 2048
N_CORES = 8
HPC = 8            # heads per core
DQ = HPC * HEAD    # 512 packed head dims per core
TB = 128           # t-block
SC = 512           # s-chunk (psum bank)
KT = HIDDEN // 128 # 8 k-tiles for the projections

F32R = mybir.dt.float32r
BF16 = mybir.dt.bfloat16
F16 = mybir.dt.float16
F32 = mybir.dt.float32


def _walk_blocks(fn):
    out = []

    def rec(b):
        out.append(b)
        for sb in getattr(b, "blocks", []) or []:
            rec(sb)

    for b in fn.blocks:
        rec(b)
    return out


def split_overloaded_waits(nc, max_waits=1):
    """Walrus (neuronxcc) rejects instructions with more than a couple of sem
    waits ("Too many sync wait commands"). Split excess waits into preceding
    same-engine NoOps — same-engine program order makes this semantically
    identical."""
    n_split = 0
    for fn in nc.m.functions:
        for bb in _walk_blocks(fn):
            insts = list(bb.instructions)
            new_insts = []
            changed = False
            for inst in insts:
                si = inst.sync_info
                waits = list(si.on_wait) if si is not None and si.on_wait else []
                if len(waits) > max_waits:
                    head, tail = waits[:-max_waits], waits[-max_waits:]
                    k = 0
                    while head:
                        chunk, head = head[:max_waits], head[max_waits:]
                        nop = mybir.InstNoOp(name=f"{inst.name}-ws{k}", ins=[], outs=[])
                        nop.engine = inst.engine
                        nop.sync_info = mybir.SyncInfo(on_wait=chunk, on_update=[])
                        new_insts.append(nop)
                        k += 1
                    inst.sync_info = mybir.SyncInfo(
                        on_wait=tail,
                        on_update=list(si.on_update) if si.on_update else [],
                    )
                    n_split += 1
                    changed = True
                new_insts.append(inst)
            if changed:
                bb.instructions = new_insts
    return n_split


def build_module(iters=1, postpass=True):
    """Build the per-core Bass program. iters>1 wraps the whole compute in a
    hardware loop for wall-clock timing (inputs re-read each iteration)."""
    nc = bass.Bass(target_bir_lowering=False)

    d_xth = nc.dram_tensor("xth", [HIDDEN, SEQ], F16, kind="ExternalInput")
    d_xtl = nc.dram_tensor("xtl", [HIDDEN, SEQ], F16, kind="ExternalInput")
    d_xtv = nc.dram_tensor("xtv", [HIDDEN, SEQ], BF16, kind="ExternalInput")
    d_wqh = nc.dram_tensor("wqh", [HIDDEN, DQ], F16, kind="ExternalInput")
    d_wql = nc.dram_tensor("wql", [HIDDEN, DQ], F16, kind="ExternalInput")
    d_wkh = nc.dram_tensor("wkh", [HIDDEN, DQ], F16, kind="ExternalInput")
    d_wkl = nc.dram_tensor("wkl", [HIDDEN, DQ], F16, kind="ExternalInput")
    d_wv = nc.dram_tensor("wv", [HIDDEN, DQ], BF16, kind="ExternalInput")
    d_ow = nc.dram_tensor("ow", [DQ, HIDDEN], BF16, kind="ExternalInput")
    d_out = nc.dram_tensor("out_p", [SEQ, HIDDEN], F32, kind="ExternalOutput")

    NTB = SEQ // TB        # 16 t-blocks
    NSB = SEQ // 128       # 16 s-blocks
    NSC = SEQ // SC        # 4 s-chunks
    NDQ = DQ // 128        # 4 dq-tiles (head pairs)

    with tile.TileContext(nc) as tc:
        def body(_iv=None):
            # ---- persistent tiles (per iteration) ----
            with tc.tile_pool(name="persist", bufs=1) as pp:
                # S = (Qhi+Qlo)^T (Khi+Klo) via two K=128 matmuls:
                #   qpk @ kpk_hl = Qhi.Khi + Qlo.Klo
                #   qpk @ kpk_lh = Qhi.Klo + Qlo.Khi
                # qpk[0:64, n, t] = Qhi_n, qpk[64:128, n, t] = Qlo_n
                qpk = pp.tile([128, HPC, SEQ], F16, name="qpk")
                kpk_hl = pp.tile([128, HPC, SEQ], F16, name="kpk_hl")
                kpk_lh = pp.tile([128, HPC, SEQ], F16, name="kpk_lh")
                vS = pp.tile([128, NSB, DQ], BF16, name="vS")
                owS = pp.tile([128, NDQ, HIDDEN], BF16, name="owS")
                sig = pp.tile([128, HPC, NTB], F32, name="sig")

                nc.scalar.dma_start(owS[:], d_ow.rearrange("(a p) e -> p a e", p=128))

                # ---- stage B: Q^T, K^T projections (fp16 hi/lo, 3 terms) ----
                with tc.tile_pool(name="bpool", bufs=2) as bp, \
                     tc.tile_pool(name="bpsum", bufs=4, space="PSUM") as bps:
                    wqh_sb = bp.tile([128, KT, DQ], F16, name="wqh_sb", bufs=1)
                    wql_sb = bp.tile([128, KT, DQ], F16, name="wql_sb", bufs=1)
                    wkh_sb = bp.tile([128, KT, DQ], F16, name="wkh_sb", bufs=1)
                    wkl_sb = bp.tile([128, KT, DQ], F16, name="wkl_sb", bufs=1)
                    for d_w, w_sb in ((d_wqh, wqh_sb), (d_wql, wql_sb),
                                      (d_wkh, wkh_sb), (d_wkl, wkl_sb)):
                        nc.scalar.dma_start(
                            w_sb[:], d_w.rearrange("(a p) e -> p a e", p=128))
                    xth_r = d_xth.rearrange("(a p) t -> p a t", p=128)
                    xtl_r = d_xtl.rearrange("(a p) t -> p a t", p=128)
                    for tc4 in range(SEQ // SC):
                        xth_c = bp.tile([128, KT, SC], F16, tag="xth_c")
                        xtl_c = bp.tile([128, KT, SC], F16, tag="xtl_c")
                        nc.scalar.dma_start(xth_c[:], xth_r[:, :, tc4 * SC:(tc4 + 1) * SC])
                        nc.scalar.dma_start(xtl_c[:], xtl_r[:, :, tc4 * SC:(tc4 + 1) * SC])
                        tsl = slice(tc4 * SC, (tc4 + 1) * SC)
                        for dqt in range(NDQ):
                            for wh_sb, wl_sb, isq in ((wqh_sb, wql_sb, True),
                                                      (wkh_sb, wkl_sb, False)):
                                ps = bps.tile([128, SC], F32, tag="bps")
                                terms = [(wh_sb, xth_c), (wh_sb, xtl_c), (wl_sb, xth_c)]
                                nmm = len(terms) * KT
                                i = 0
                                for w_sb, x_c in terms:
                                    for kt in range(KT):
                                        nc.tensor.matmul(
                                            ps[:],
                                            w_sb[:, kt, dqt * 128:(dqt + 1) * 128],
                                            x_c[:, kt, :],
                                            start=(i == 0), stop=(i == nmm - 1),
                                        )
                                        i += 1
                                # split psum into fp16 hi/lo for the S matmuls
                                for par in range(2):
                                    n = dqt * 2 + par
                                    po = par * 64
                                    if isq:
                                        nc.scalar.copy(qpk[0:64, n, tsl], ps[po:po + 64, :])
                                        nc.vector.tensor_tensor(
                                            qpk[64:128, n, tsl], ps[po:po + 64, :],
                                            qpk[0:64, n, tsl],
                                            op=mybir.AluOpType.subtract)
                                    else:
                                        nc.scalar.copy(kpk_hl[0:64, n, tsl], ps[po:po + 64, :])
                                        nc.vector.tensor_tensor(
                                            kpk_hl[64:128, n, tsl], ps[po:po + 64, :],
                                            kpk_hl[0:64, n, tsl],
                                            op=mybir.AluOpType.subtract)
                                        nc.scalar.copy(kpk_lh[64:128, n, tsl],
                                                       kpk_hl[0:64, n, tsl])
                                        nc.vector.tensor_copy(
                                            kpk_lh[0:64, n, tsl],
                                            kpk_hl[64:128, n, tsl])

                # ---- stage C: V projection ----
                with tc.tile_pool(name="cpool", bufs=2) as cp, \
                     tc.tile_pool(name="cpsum", bufs=4, space="PSUM") as cps:
                    wv_sb = cp.tile([128, KT, DQ], BF16, name="wv_sb", bufs=1)
                    nc.scalar.dma_start(wv_sb[:], d_wv.rearrange("(a p) e -> p a e", p=128))
                    xtv_r = d_xtv.rearrange("(a p) t -> p a t", p=128)
                    for sb in range(NSB):
                        xtv_c = cp.tile([128, KT, 128], BF16, tag="xtv_c")
                        nc.scalar.dma_start(xtv_c[:], xtv_r[:, :, sb * 128:(sb + 1) * 128])
                        ps = cps.tile([128, DQ], F32, tag="cps")
                        for kt in range(KT):
                            nc.tensor.matmul(
                                ps[:], xtv_c[:, kt, :], wv_sb[:, kt, :],
                                start=(kt == 0), stop=(kt == KT - 1))
                        nc.scalar.copy(vS[:, sb, :], ps[:])

                # ---- stages D+E share ctN ----
                with tc.tile_pool(name="depool", bufs=1) as dep:
                  ctN = dep.tile([128, NDQ, SEQ], BF16, name="ctN")
                  # ---- stage D: attention ----
                  with tc.tile_pool(name="dpool", bufs=1) as dp, \
                       tc.tile_pool(name="dpsum", bufs=1, space="PSUM") as dps:
                    ctraw = {}
                    cn = {}
                    for n in range(HPC):
                        dqt, poff = n // 2, (n % 2) * 64
                        if n % 2 == 0:
                            ctraw[dqt] = dp.tile([128, SEQ], BF16, tag="ctraw", bufs=2, name=f"ctraw{dqt}")
                            cn[dqt] = dp.tile([128, NTB, 2, HEAD], BF16, tag="cn", bufs=2, name=f"cn{dqt}")
                        for j in range(NTB):
                            halves = []
                            negs = []
                            for h in range(2):
                                s_ps = dps.tile([128, SEQ // 2], F32,
                                                tag="s_ps", bufs=3, name=f"s_ps")
                                for sc in range(2):
                                    off = (2 * h + sc) * SC
                                    nc.tensor.matmul(
                                        s_ps[:, sc * SC:(sc + 1) * SC],
                                        qpk[:, n, j * TB:(j + 1) * TB],
                                        kpk_hl[:, n, off:off + SC],
                                        start=True, stop=False)
                                    nc.tensor.matmul(
                                        s_ps[:, sc * SC:(sc + 1) * SC],
                                        qpk[:, n, j * TB:(j + 1) * TB],
                                        kpk_lh[:, n, off:off + SC],
                                        start=False, stop=True)
                                nm = dp.tile([128, 1], F32, tag="negm", bufs=8,
                                             name="nm")
                                nc.vector.tensor_reduce(
                                    nm[:], s_ps[:], axis=mybir.AxisListType.X,
                                    op=mybir.AluOpType.max, negate=True)
                                halves.append(s_ps)
                                negs.append(nm)
                            negm = dp.tile([128, 1], F32, tag="negmc", bufs=4)
                            nc.vector.tensor_tensor(
                                negm[:], negs[0][:], negs[1][:],
                                op=mybir.AluOpType.min)
                            e_sb = dp.tile([128, SEQ], BF16, tag="e_sb", bufs=3)
                            sgh = dp.tile([128, 2], F32, tag="sgh", bufs=4)
                            for h in range(2):
                                nc.scalar.activation(
                                    e_sb[:, h * (SEQ // 2):(h + 1) * (SEQ // 2)],
                                    halves[h][:],
                                    mybir.ActivationFunctionType.Exp,
                                    bias=negm[:], accum_out=sgh[:, h:h + 1])
                            nc.vector.tensor_tensor(
                                sig[:, n, j:j + 1], sgh[:, 0:1], sgh[:, 1:2],
                                op=mybir.AluOpType.add)
                            if j % 4 == 0:
                                eT = dp.tile([128, NSB, SC], BF16, tag="eT", bufs=2)
                            nc.sync.dma_start(
                                eT[:, :, (j % 4) * TB:(j % 4 + 1) * TB], e_sb[:],
                                transpose=True)
                            if j % 4 == 3:
                                tc4 = j // 4
                                ct_ps = dps.tile([64, SC], F32, tag="ct_ps", bufs=2)
                                for sb in range(NSB):
                                    nc.tensor.matmul(
                                        ct_ps[:], vS[:, sb, n * HEAD:(n + 1) * HEAD],
                                        eT[:, sb, :],
                                        start=(sb == 0), stop=(sb == NSB - 1))
                                nc.scalar.copy(
                                    ctraw[dqt][poff:poff + 64, tc4 * SC:(tc4 + 1) * SC],
                                    ct_ps[:])
                        # head epilogue: rho = 1/sigma; C = T(C^T); C *= rho
                        rho = dp.tile([128, NTB], F32, tag="rho", bufs=2)
                        nc.vector.reciprocal(rho[:], sig[:, n, :])
                        craw = dp.tile([128, NTB, HEAD], BF16, tag="craw", bufs=2)
                        nc.sync.dma_start(
                            craw[:], ctraw[dqt][poff:poff + 64, :], transpose=True)
                        for j in range(NTB):
                            nc.vector.tensor_scalar_mul(
                                cn[dqt][:, j, n % 2, :], craw[:, j, :], rho[:, j:j + 1])
                        if n % 2 == 1:
                            for j in range(NTB):
                                nc.sync.dma_start(
                                    ctN[:, dqt, j * TB:(j + 1) * TB],
                                    cn[dqt][:, j, :, :], transpose=True)

                  # ---- stage E: output projection ----
                  with tc.tile_pool(name="epool", bufs=2) as ep, \
                       tc.tile_pool(name="epsum", bufs=2, space="PSUM") as eps:
                    for j in range(NTB):
                        o_sb = ep.tile([128, HIDDEN], F32, tag="o_sb")
                        for ec in range(HIDDEN // SC):
                            o_ps = eps.tile([128, SC], F32, tag="o_ps")
                            for dqt in range(NDQ):
                                nc.tensor.matmul(
                                    o_ps[:],
                                    ctN[:, dqt, j * TB:(j + 1) * TB],
                                    owS[:, dqt, ec * SC:(ec + 1) * SC],
                                    start=(dqt == 0), stop=(dqt == NDQ - 1))
                            nc.vector.tensor_copy(o_sb[:, ec * SC:(ec + 1) * SC], o_ps[:])
                        nc.scalar.dma_start(d_out[j * TB:(j + 1) * TB, :], o_sb[:])

        if iters == 1:
            body()
        else:
            with tc.For_i(0, iters, 1) as iv:
                body(iv)

    if postpass:
        split_overloaded_waits(nc)
    return nc


def shard_inputs(x, qkv, out_weight):
    """Host-side sharding: per-core input dicts."""
    x = np.ascontiguousarray(np.asarray(x, dtype=np.float32))
    qkv = np.ascontiguousarray(np.asarray(qkv, dtype=np.float32))
    ow = np.asarray(out_weight, dtype=np.float32) / np.sqrt(np.float32(HEAD))
    in_maps = []
    for c in range(N_CORES):
        b, hg = c // 2, c % 2
        cols = slice(hg * DQ, (hg + 1) * DQ)
        xt = np.ascontiguousarray(x[b].T)                      # [1024, 2048]
        wq = np.ascontiguousarray(qkv[:, 0:HIDDEN][:, cols])
        wk = np.ascontiguousarray(qkv[:, HIDDEN:2 * HIDDEN][:, cols])
        wv = np.ascontiguousarray(qkv[:, 2 * HIDDEN:][:, cols])
        owc = np.ascontiguousarray(ow[hg * DQ:(hg + 1) * DQ, :])

        def split16(a):
            hi = a.astype(np.float16)
            lo = (a - hi.astype(np.float32)).astype(np.float16)
            return hi, lo

        xth, xtl = split16(xt)
        wqh, wql = split16(wq)
        wkh, wkl = split16(wk)
        in_maps.append({
            "xth": xth,
            "xtl": xtl,
            "xtv": xt.astype(ml_dtypes.bfloat16),
            "wqh": wqh,
            "wql": wql,
            "wkh": wkh,
            "wkl": wkl,
            "wv": wv.astype(ml_dtypes.bfloat16),
            "ow": owc.astype(ml_dtypes.bfloat16),
        })
    return in_maps


_CACHED = {}


def get_module(iters=1):
    if iters not in _CACHED:
        _CACHED[iters] = build_module(iters)
    return _CACHED[iters]


def run_sharded(in_maps, iters=1):
    nc = get_module(iters)
    res = run_bass_kernel_spmd(nc, in_maps, core_ids=list(range(N_CORES)))
    return res


def kernel(x, qkv, out_weight):
    in_maps = shard_inputs(x, qkv, out_weight)
    res = run_sharded(in_maps)
    out = np.empty((BATCH, SEQ, HIDDEN), dtype=np.float32)
    for b in range(BATCH):
        out[b] = res.results[2 * b]["out_p"] + res.results[2 * b + 1]["out_p"]
    return out
